# revision 55
# baseline (speedup 1.0000x reference)
"""Trainium2 Bass kernel for nn_ConvLTVFilterGenerator.

Pipeline (per batch elem, data-parallel over B across 8 cores, 2 elems/core):
  conv stack (f32r; L1 tap-packed into 2 K-chunks; L4 bias-add on DVE)
  -> cepstrum DFT rey/imy (f32r matmuls; frequencies permuted by k mod 4)
  -> Z-1 ~= u + i*phi (1st-order suffices: 2nd-order terms sit below the
     fp16 noise floor of the stored spectra)
  -> frame spectra via half-block DFTs: frames overlap 50%, so compute
     256-sample block DFTs U (cos) / W (sin) once per block (fp16 matmuls
     against even/odd sample planes VE/VO, contiguous rhs) and assemble
     Fr = U_t +- {U,W}_{t+1}, Fi = -(W_t -+ {U,W}_{t+1}); the k mod 4
     frequency grouping makes the combine a single uniform add/sub per
     128-row block. Halves the frame-DFT matmul work and PSUM->SBUF copies.
  -> products pre/pim (f32 intermediates, fp16 result) -> out-stage fp16
     matmuls with the OLA fold and k=512/identity rows accumulated in PSUM
  -> clip on DVE, per-block output DMAs.
Engine split: PE matmuls; Act relu + PSUM->SBUF copies (u/phi/U); DVE
elementwise products/combines/W-copies/clip; SP input + output DMAs.
Loop mode: For_i carries an all-engine barrier, so the body is unrolled 4x
inside the loop to overlap iterations.
"""
import sys

sys.path.insert(0, "/opt/trn_rl_repo")

import numpy as np

import concourse.bacc as bacc
import concourse.tile as tile
from concourse import mybir
from concourse.bass_utils import run_bass_kernel_spmd

B, T, D = 16, 800, 80
HOP, WIN, FFT = 256, 512, 1024
CCH, OUT, GRP = 256, 222, 8
NK = FFT // 2 + 1          # 513
N_CORES = 8
BPC = B // N_CORES         # 2 batch elems per core
ZPAD = T * HOP + 512       # 205312 = 1604*128
NU = ZPAD // 128           # 1604
NB = NU // 2               # 802 256-sample blocks (801 used)
F = 400                    # frames per half

f32 = mybir.dt.float32
f32r = mybir.dt.float32r
f16 = mybir.dt.float16
AF = mybir.ActivationFunctionType
ALU = mybir.AluOpType

# residue-block combine tables (see module docstring)
#   Fr_r = U_t op Z_{t+1}; G_r = W_t op Z'_{t+1}; Fi = -G
FR_TAB = [(ALU.add, "U"), (ALU.subtract, "W"), (ALU.subtract, "U"), (ALU.add, "W")]
G_TAB = [(ALU.add, "W"), (ALU.add, "U"), (ALU.subtract, "W"), (ALU.subtract, "U")]

_MATS = None
_NCS = {}


def _build_matrices():
    """Input-independent DFT/OLA matrices, host-side fp64 -> fp32/fp16."""
    global _MATS
    if _MATS is not None:
        return _MATS
    w = 2 * np.pi / FFT
    pad = (FFT - OUT) // 2
    s_exp = np.log(10.0) / 10.0
    kperm = np.concatenate([np.arange(r, 512, 4) for r in range(4)])
    kfull = np.concatenate([kperm, [512]]).astype(np.float64)   # (513,)
    c = np.arange(OUT, dtype=np.float64)[None, :]
    CaN = np.cos(w * kfull[:, None] * (pad + c)) * s_exp        # (513, 222)
    SaN = -np.sin(w * kfull[:, None] * (pad + c))

    # cepstrum DFT lhsT: (128, 2, 513) rows = cepstral idx, cols = perm freq
    cat = np.zeros((128, 2, NK), np.float64)
    sat = np.zeros((128, 2, NK), np.float64)
    for ch in range(2):
        rows = min(128, OUT - 128 * ch)
        cat[:rows, ch, :] = CaN[:, 128 * ch:128 * ch + rows].T
        sat[:rows, ch, :] = SaN[:, 128 * ch:128 * ch + rows].T

    # block-DFT lhsT: cau[nn, r, ch, p] = cos(w*(4p+r)*(128*ch+nn))
    cau = np.zeros((128, 4, 2, 128), np.float64)
    sau = np.zeros((128, 4, 2, 128), np.float64)
    nn = np.arange(128, dtype=np.float64)
    for r in range(4):
        K = np.arange(r, 512, 4, dtype=np.float64)[None, :]     # (1,128)
        for ch in range(2):
            nf = (128 * ch + nn)[:, None]
            cau[:, r, ch, :] = np.cos(w * K * nf)
            sau[:, r, ch, :] = np.sin(w * K * nf)
    ca5u = np.zeros((128, 2, 1), np.float64)
    for ch in range(2):
        ca5u[:, ch, 0] = np.cos(np.pi * (128 * ch + nn))        # (-1)^n

    # out-stage matrices (freq-permuted rows)
    n = np.arange(WIN, dtype=np.float64)
    win = 0.5 * (1.0 - np.cos(2.0 * np.pi * n / WIN))
    wk = np.full(NK, 2.0)
    wk[0] = 1.0
    wk[-1] = 1.0
    dd = (WIN - 1 - n)[None, :]
    kf = np.arange(NK, dtype=np.float64)[:, None]
    C2 = (win[None, :] * wk[:, None] * np.cos(w * kf * dd)) / FFT   # (513,512)
    S2 = (-win[None, :] * wk[:, None] * np.sin(w * kf * dd)) / FFT
    c2a = np.zeros((128, 4, HOP), np.float64)
    c2b = np.zeros((128, 4, HOP), np.float64)
    s2a = np.zeros((128, 4, HOP), np.float64)
    s2b = np.zeros((128, 4, HOP), np.float64)
    p = np.arange(128)
    for r in range(4):
        krows = 4 * p + r
        c2a[:, r, :] = C2[krows, :HOP]
        c2b[:, r, :] = C2[krows, HOP:]
        s2a[:, r, :] = S2[krows, :HOP]
        s2b[:, r, :] = S2[krows, HOP:]
    c2a4 = C2[512:513, :HOP]
    c2b4 = C2[512:513, HOP:]
    c2b5 = np.zeros((1, HOP), np.float64)
    c2b5[0, HOP - 1] = win[-1]

    def a32(a):
        return np.ascontiguousarray(a, np.float32)

    def a16(a):
        return np.ascontiguousarray(a, np.float16)

    _MATS = dict(
        cat=a32(cat), sat=a32(sat),
        cau=a16(cau), sau=a16(sau), ca5u=a16(ca5u),
        c2a=a16(c2a), c2b=a16(c2b), s2a=a16(s2a), s2b=a16(s2b),
        c2a4=a16(c2a4), c2b4=a16(c2b4), c2b5=a16(c2b5))
    return _MATS


def _prep_weights(inp):
    """Input-dependent weight rearrangements (host). Same as baseline."""
    idx = np.arange(1, OUT // 2 + 1, dtype=np.float64)
    quef = np.concatenate([idx[::-1], idx])
    W1 = np.asarray(inp["W1"], np.float64)
    W2 = np.asarray(inp["W2"], np.float64)
    W3 = np.asarray(inp["W3"], np.float64)
    W4 = np.asarray(inp["W4"], np.float64)
    # L1 tap-packed lhsT: contraction over (cin, tap) in two K-chunks.
    # x3a rows: [x shift0 (cin 0..79); x shift1 (cin 0..47)]
    # x3b rows: [x shift1 (cin 48..79); x shift2 (cin 0..79)]
    w13a = np.zeros((128, 2, 128), np.float64)
    w13b = np.zeros((112, 2, 128), np.float64)
    for m in range(2):
        co = slice(128 * m, 128 * (m + 1))
        w13a[0:80, m, :] = W1[co, :, 0].T
        w13a[80:128, m, :] = W1[co, 0:48, 1].T
        w13b[0:32, m, :] = W1[co, 48:80, 1].T
        w13b[32:112, m, :] = W1[co, :, 2].T

    def blockdiag(W):
        bd = np.zeros((128, 3, 2, 128), np.float64)
        for H in range(2):
            for o in range(128):
                g = o // 32
                for kk in range(3):
                    bd[g * 32:(g + 1) * 32, kk, H, o] = W[128 * H + o, :, kk]
        return np.ascontiguousarray(bd, np.float32)

    W4q = W4 / quef[:, None, None]
    w4t = np.zeros((128, 2, 3, OUT), np.float64)
    for cch in range(2):
        for kk in range(3):
            w4t[:, cch, kk, :] = W4q[:, 128 * cch:128 * (cch + 1), kk].T
    b4q = np.asarray(inp["b4"], np.float64) / quef

    def bias2(b):
        out = np.zeros((128, 2), np.float32)
        bb = np.asarray(b, np.float64)
        out[:, 0] = bb[:128]
        out[:len(bb) - 128, 1] = bb[128:]
        return out

    return dict(
        w13a=np.ascontiguousarray(w13a, np.float32),
        w13b=np.ascontiguousarray(w13b, np.float32),
        bd2=blockdiag(W2), bd3=blockdiag(W3),
        w4t=np.ascontiguousarray(w4t, np.float32),
        b1t=bias2(inp["b1"]), b2t=bias2(inp["b2"]), b3t=bias2(inp["b3"]),
        b4t=bias2(b4q))


def build_nc(loop_n=1):
    """Build + compile the per-core Bass program."""
    if loop_n in _NCS:
        return _NCS[loop_n]
    nc = bacc.Bacc("TRN2", target_bir_lowering=False, debug=False)

    def din(name, shape, dt=f32r):
        return nc.dram_tensor(name, list(shape), dt, kind="ExternalInput").ap()

    XT = din("xt", (BPC, D, T + 2))
    VE = din("ve", (BPC, 128, NB), f16)
    VO = din("vo", (BPC, 128, NB), f16)
    CAT = din("cat", (128, 2, NK))
    SAT = din("sat", (128, 2, NK))
    CAU = din("cau", (128, 4, 2, 128), f16)
    SAU = din("sau", (128, 4, 2, 128), f16)
    CA5U = din("ca5u", (128, 2, 1), f16)
    C2A = din("c2a", (128, 4, HOP), f16)
    C2B = din("c2b", (128, 4, HOP), f16)
    S2A = din("s2a", (128, 4, HOP), f16)
    S2B = din("s2b", (128, 4, HOP), f16)
    C2A4 = din("c2a4", (1, HOP), f16)
    C2B4 = din("c2b4", (1, HOP), f16)
    C2B5 = din("c2b5", (1, HOP), f16)
    W13A = din("w13a", (128, 2, 128))
    W13B = din("w13b", (112, 2, 128))
    BD2 = din("bd2", (128, 3, 2, 128))
    BD3 = din("bd3", (128, 3, 2, 128))
    W4T = din("w4t", (128, 2, 3, OUT))
    B1 = nc.dram_tensor("b1t", [128, 2], f32, kind="ExternalInput").ap()
    B2 = nc.dram_tensor("b2t", [128, 2], f32, kind="ExternalInput").ap()
    B3 = nc.dram_tensor("b3t", [128, 2], f32, kind="ExternalInput").ap()
    B4 = nc.dram_tensor("b4t", [128, 2], f32, kind="ExternalInput").ap()
    OUTD = nc.dram_tensor("out", [BPC, T, HOP], f32, kind="ExternalOutput").ap()

    with tile.TileContext(nc) as tc:
        with tc.tile_pool(name="consts", bufs=1) as cst, \
             tc.tile_pool(name="data", bufs=2) as dat, \
             tc.tile_pool(name="work", bufs=3) as wk, \
             tc.tile_pool(name="psum", bufs=2, space="PSUM") as ps:

            def load(name, src, shape, dt=f32r):
                t = cst.tile(list(shape), dt, name=name)
                nc.sync.dma_start(out=t, in_=src)
                return t

            # conv-critical weights first so the first matmul isn't gated
            # on the long const-DMA queue
            w13a = load("w13aS", W13A, (128, 2, 128))
            w13b = load("w13bS", W13B, (112, 2, 128))
            b1t = load("b1tS", B1, (128, 2), f32)
            bd2 = load("bd2S", BD2, (128, 3, 2, 128))
            bd3 = load("bd3S", BD3, (128, 3, 2, 128))
            b2t = load("b2tS", B2, (128, 2), f32)
            b3t = load("b3tS", B3, (128, 2), f32)
            cat = load("catS", CAT, (128, 2, NK))
            sat = load("satS", SAT, (128, 2, NK))
            cau = load("cauS", CAU, (128, 4, 2, 128), f16)
            sau = load("sauS", SAU, (128, 4, 2, 128), f16)
            ca5u = load("ca5uS", CA5U, (128, 2, 1), f16)
            c2a = load("c2aS", C2A, (128, 4, HOP), f16)
            c2b = load("c2bS", C2B, (128, 4, HOP), f16)
            s2a = load("s2aS", S2A, (128, 4, HOP), f16)
            s2b = load("s2bS", S2B, (128, 4, HOP), f16)
            c2a4 = load("c2a4S", C2A4, (1, HOP), f16)
            c2b4 = load("c2b4S", C2B4, (1, HOP), f16)
            c2b5 = load("c2b5S", C2B5, (1, HOP), f16)
            w4t = load("w4tS", W4T, (128, 2, 3, OUT))
            b4t = load("b4tS", B4, (128, 2), f32)
            zb = cst.tile([128, 1], f32, name="zb")
            nc.vector.memset(zb, 0.0)

            # h tiles live outside the loop (bufs=1): their halo columns are
            # zeroed once here and never touched again (relu writes 1..T only)
            h1 = dat.tile([128, 2, T + 2], f32r, tag="h1", name="h1", bufs=1)
            h2 = dat.tile([128, 2, T + 2], f32r, tag="h2", name="h2", bufs=1)
            h3 = dat.tile([128, 2, T + 2], f32r, tag="h3", name="h3", bufs=1)
            for h in (h1, h2, h3):
                for m in range(2):
                    nc.vector.tensor_copy(h[:, m, 0:1], zb)
                    nc.vector.tensor_copy(h[:, m, T + 1:T + 2], zb)

            def body():
                elems = []
                xv = []
                for b in range(BPC):
                    # ---- input DMAs for both elems up-front (x3 chunks
                    # first: the first conv matmul gates on them)
                    x3a = dat.tile([128, T], f32r, tag="x3a", name="x3a")
                    x3b = dat.tile([112, T], f32r, tag="x3b", name="x3b")
                    nc.sync.dma_start(out=x3a[0:80, 0:T], in_=XT[b, :, 0:T])
                    nc.sync.dma_start(out=x3a[80:128, 0:T], in_=XT[b, 0:48, 1:T + 1])
                    nc.sync.dma_start(out=x3b[0:32, 0:T], in_=XT[b, 48:80, 1:T + 1])
                    nc.sync.dma_start(out=x3b[32:112, 0:T], in_=XT[b, :, 2:T + 2])
                    vE = dat.tile([128, NB], f16, tag="ve", name="vE")
                    nc.sync.dma_start(out=vE, in_=VE[b])
                    vO = dat.tile([128, NB], f16, tag="vo", name="vO")
                    nc.sync.dma_start(out=vO, in_=VO[b])
                    xv.append((x3a, x3b, vE, vO))

                for b in range(BPC):
                    x3a, x3b, vE, vO = xv[b]
                    # ---- conv stack (L1 tap-packed; L4 bias-add on DVE)
                    ccep = dat.tile([128, 2, T], f32r, tag="ccep", name="ccep")

                    for t0 in (0, F):
                        # interleave the m0/m1 accumulation chains so
                        # consecutive matmuls hit different PSUM banks
                        # (chained matmuls into one bank stall ~270ns on HW)
                        pcs = [ps.tile([128, F], f32, tag="ri", name="pc1",
                                       bufs=3) for _ in range(2)]
                        for w13, last in ((w13a, False), (w13b, True)):
                            for m in range(2):
                                nc.tensor.matmul(
                                    pcs[m], w13[:, m, :],
                                    (x3a if not last else x3b)[:, t0:t0 + F],
                                    start=not last, stop=last)
                        for m in range(2):
                            nc.scalar.activation(
                                h1[:, m, 1 + t0:1 + t0 + F], pcs[m], AF.Relu,
                                bias=b1t[:, m:m + 1], scale=1.0)
                    for hsrc, hdst, bdw, bt in ((h1, h2, bd2, b2t), (h2, h3, bd3, b3t)):
                        for t0 in (0, F):
                            pcs = [ps.tile([128, F], f32, tag="ri", name="pc2",
                                           bufs=3) for _ in range(2)]
                            for kk in range(3):
                                for m in range(2):
                                    nc.tensor.matmul(
                                        pcs[m], bdw[:, kk, m, :],
                                        hsrc[:, m, t0 + kk:t0 + kk + F],
                                        start=(kk == 0), stop=(kk == 2))
                            for m in range(2):
                                nc.scalar.activation(
                                    hdst[:, m, 1 + t0:1 + t0 + F], pcs[m],
                                    AF.Relu, bias=bt[:, m:m + 1], scale=1.0)
                    for t0 in (0, F):
                        pcs = [ps.tile([128, F], f32, tag="ri", name="pc4",
                                       bufs=3) for _ in range(2)]
                        for i, (cch, kk) in enumerate(
                                [(c, k) for c in range(2) for k in range(3)]):
                            for m in range(2):
                                sz = min(128, OUT - 128 * m)
                                nc.tensor.matmul(
                                    pcs[m][:sz],
                                    w4t[:, cch, kk, 128 * m:128 * m + sz],
                                    h3[:, cch, t0 + kk:t0 + kk + F],
                                    start=(i == 0), stop=(i == 5))
                        for m in range(2):
                            sz = min(128, OUT - 128 * m)
                            nc.vector.tensor_scalar_add(
                                ccep[:sz, m, t0:t0 + F], pcs[m][:sz],
                                b4t[:sz, m:m + 1])

                    # ---- spectral stage per (t0, r); U/W block DFTs fused
                    # into the t0=0 pass for pipelining
                    usb, wsb = [], []
                    for r in range(4):
                        usb.append(dat.tile([128, NB - 1], f16, tag=f"usb{r}",
                                            name=f"usb{r}"))
                        wsb.append(dat.tile([128, NB - 1], f16, tag=f"wsb{r}",
                                            name=f"wsb{r}"))
                    u5sb = dat.tile([1, NB - 1], f16, tag="u5sb", name="u5sb")
                    pre = [dat.tile([128, T + 1], f16, tag=f"pre{r}",
                                    name=f"pre{r}") for r in range(4)]
                    pim = [dat.tile([128, T + 1], f16, tag=f"pim{r}",
                                    name=f"pim{r}") for r in range(4)]
                    pre5 = dat.tile([1, T + 1], f16, tag="pre5", name="pre5")

                    for t0 in (0, F):
                        for r in range(4):
                            if t0 == 0:
                                # block DFTs U/W -> PSUM (PE); U and W chains
                                # interleaved across PSUM banks; copies to
                                # SBUF fp16 (U on Act, W on DVE for balance)
                                for half in range(2):
                                    ncols = 401 if half == 0 else 400
                                    j0 = 401 * half
                                    puU = ps.tile([128, 401], f32, tag="uw",
                                                  name="puU", bufs=2)
                                    puW = ps.tile([128, 401], f32, tag="uw",
                                                  name="puW", bufs=2)
                                    for ch, vv in ((0, vE), (1, vO)):
                                        rhs = vv[:, j0:j0 + ncols]
                                        nc.tensor.matmul(
                                            puU[:, :ncols], cau[:, r, ch, :],
                                            rhs, start=(ch == 0),
                                            stop=(ch == 1))
                                        nc.tensor.matmul(
                                            puW[:, :ncols], sau[:, r, ch, :],
                                            rhs, start=(ch == 0),
                                            stop=(ch == 1))
                                    nc.scalar.activation(
                                        usb[r][:, j0:j0 + ncols],
                                        puU[:, :ncols], AF.Copy)
                                    nc.vector.tensor_copy(
                                        wsb[r][:, j0:j0 + ncols],
                                        puW[:, :ncols])
                            ks = slice(128 * r, 128 * (r + 1))
                            rey = ps.tile([128, F], f32, tag="ri", name="rey", bufs=3)
                            nc.tensor.matmul(rey, cat[:, 0, ks],
                                             ccep[:, 0, t0:t0 + F],
                                             start=True, stop=False)
                            nc.tensor.matmul(rey, cat[:94, 1, ks],
                                             ccep[:94, 1, t0:t0 + F],
                                             start=False, stop=True)
                            imy = ps.tile([128, F], f32, tag="ri", name="imy", bufs=3)
                            nc.tensor.matmul(imy, sat[:, 0, ks],
                                             ccep[:, 0, t0:t0 + F],
                                             start=True, stop=False)
                            nc.tensor.matmul(imy, sat[:94, 1, ks],
                                             ccep[:94, 1, t0:t0 + F],
                                             start=False, stop=True)
                            # Z-1 ~= u + i*phi (1st order suffices: 2nd-order
                            # terms are below the fp16 noise floor)
                            a1 = wk.tile([128, F], f32, tag="uS", name="uS")
                            b_t = wk.tile([128, F], f32, tag="phiS", name="phiS")
                            nc.scalar.activation(a1, rey, AF.Copy)
                            nc.scalar.activation(b_t, imy, AF.Copy)
                            # Fr = U_t op Z_{t+1} ; G = W_t op Z'_{t+1}  (Pool)
                            fop, fz = FR_TAB[r]
                            gop, gz = G_TAB[r]
                            Zf = usb[r] if fz == "U" else wsb[r]
                            Zg = usb[r] if gz == "U" else wsb[r]
                            fr = wk.tile([128, F], f32, tag="fr", name="fr")
                            gg = wk.tile([128, F], f32, tag="gg", name="gg")
                            nc.vector.tensor_tensor(
                                fr, usb[r][:, t0:t0 + F],
                                Zf[:, t0 + 1:t0 + 1 + F], fop)
                            nc.vector.tensor_tensor(
                                gg, wsb[r][:, t0:t0 + F],
                                Zg[:, t0 + 1:t0 + 1 + F], gop)
                            # pre = a1*Fr - b*G ; pim = b*Fr + a1*G
                            tm1 = wk.tile([128, F], f32, tag="tm1", name="tm1")
                            tm2 = wk.tile([128, F], f32, tag="tm2", name="tm2")
                            nc.vector.tensor_mul(tm1, a1, fr)
                            nc.vector.tensor_mul(tm2, b_t, gg)
                            nc.vector.tensor_sub(
                                pre[r][:, 1 + t0:1 + t0 + F], tm1, tm2)
                            tm3 = wk.tile([128, F], f32, tag="tm3", name="tm3")
                            tm4 = wk.tile([128, F], f32, tag="tm4", name="tm4")
                            nc.vector.tensor_mul(tm3, b_t, fr)
                            nc.vector.tensor_mul(tm4, a1, gg)
                            nc.vector.tensor_add(
                                pim[r][:, 1 + t0:1 + t0 + F], tm3, tm4)

                        if t0 == 0:
                            # k512 block DFT (U only; W row is zero)
                            for half in range(2):
                                ncols = 401 if half == 0 else 400
                                j0 = 401 * half
                                pu5 = ps.tile([1, 401], f32, tag="k5",
                                              name="pu5", bufs=1)
                                for ch, vv in ((0, vE), (1, vO)):
                                    rhs = vv[:, j0:j0 + ncols]
                                    nc.tensor.matmul(pu5[:, :ncols],
                                                     ca5u[:, ch, :], rhs,
                                                     start=(ch == 0),
                                                     stop=(ch == 1))
                                nc.scalar.activation(
                                    u5sb[:, 401 * half:401 * half + ncols],
                                    pu5[:, :ncols], AF.Copy)
                        # k512 row: phi=0 so pre5 = (u5 + u5^2/2) * Fr5
                        rey5 = ps.tile([1, F], f32, tag="k5", name="rey5", bufs=1)
                        nc.tensor.matmul(rey5, cat[:, 0, 512:513],
                                         ccep[:, 0, t0:t0 + F],
                                         start=True, stop=False)
                        nc.tensor.matmul(rey5, cat[:94, 1, 512:513],
                                         ccep[:94, 1, t0:t0 + F],
                                         start=False, stop=True)
                        u5 = wk.tile([1, F], f32, tag="u5", name="u5")
                        nc.scalar.activation(u5, rey5, AF.Copy)
                        fr5 = wk.tile([1, F], f32, tag="fr5", name="fr5")
                        nc.vector.tensor_add(fr5, u5sb[:, t0:t0 + F],
                                             u5sb[:, t0 + 1:t0 + 1 + F])
                        nc.vector.tensor_mul(pre5[:, 1 + t0:1 + t0 + F], u5, fr5)

                    # identity row: frame-start samples
                    p5b = dat.tile([1, T + 1], f16, tag="p5b", name="p5b")
                    nc.vector.tensor_copy(p5b[:, 1:T + 1], vE[0:1, 0:T])
                    nc.vector.tensor_copy(p5b[:, 0:1], vE[0:1, T - 1:T])
                    # wrap halo: col 0 <- col T
                    for pt_ in pre + pim + [pre5]:
                        nc.vector.tensor_copy(pt_[:, 0:1], pt_[:, T:T + 1])

                    elems.append((pre, pim, pre5, p5b))

                # ---- out stage per elem (emitted after both spectral passes)
                for b in range(BPC):
                    pre, pim, pre5, p5b = elems[b]
                    osb = dat.tile([128, 7, HOP], f32, tag="osb", name="osb")
                    # blk 0 last (its c2b half reads the wrap column); blocks
                    # processed in pairs with their 19-matmul accumulation
                    # chains interleaved so consecutive matmuls target
                    # different PSUM banks (chained matmuls into one bank
                    # stall ~270ns on HW)
                    for blk in (1, 2, 3, 4, 5, 6, 0):
                        off = blk * 128
                        tb = min(128, T - off)
                        po = ps.tile([128, HOP], f32, tag="out", name="po", bufs=2)
                        first = True
                        for r in range(4):
                            nc.tensor.matmul(po[:tb], pre[r][:, 1 + off:1 + off + tb],
                                             c2a[:, r, :], start=first, stop=False)
                            first = False
                        for r in range(4):
                            nc.tensor.matmul(po[:tb], pim[r][:, 1 + off:1 + off + tb],
                                             s2a[:, r, :], start=False, stop=False)
                        for r in range(4):
                            nc.tensor.matmul(po[:tb], pre[r][:, off:off + tb],
                                             c2b[:, r, :], start=False, stop=False)
                        for r in range(4):
                            nc.tensor.matmul(po[:tb], pim[r][:, off:off + tb],
                                             s2b[:, r, :], start=False, stop=False)
                        nc.tensor.matmul(po[:tb], pre5[:, 1 + off:1 + off + tb],
                                         c2a4, start=False, stop=False)
                        nc.tensor.matmul(po[:tb], pre5[:, off:off + tb],
                                         c2b4, start=False, stop=False)
                        nc.tensor.matmul(po[:tb], p5b[:, off:off + tb],
                                         c2b5, start=False, stop=True)
                        nc.vector.tensor_scalar(osb[:tb, blk, :], po[:tb],
                                                1.0, -1.0, ALU.min, ALU.max)
                        nc.sync.dma_start(out=OUTD[b, off:off + tb, :],
                                          in_=osb[:tb, blk, :])

            if loop_n == 1:
                body()
            else:
                # unroll inside the hardware loop: For_i has an all-engine
                # barrier at the back-edge, so only unrolled copies overlap
                unroll = 4 if loop_n % 4 == 0 else 1
                with tc.For_i(0, loop_n // unroll, 1):
                    for _ in range(unroll):
                        body()

    nc.compile()
    _NCS[loop_n] = nc
    return nc


def _make_in_maps(inputs):
    mats = _build_matrices()
    wts = _prep_weights(inputs)
    x = np.asarray(inputs["x"], np.float32)
    z = np.asarray(inputs["z"], np.float32).reshape(B, -1)
    xt = np.zeros((B, D, T + 2), np.float32)                      # halo cols 0
    xt[:, :, 1:T + 1] = x.transpose(0, 2, 1)
    zp = np.zeros((B, ZPAD), np.float32)
    zp[:, WIN // 2 - 1:WIN // 2 - 1 + T * HOP] = z
    q = zp.reshape(B, NB, 256)
    ve = np.ascontiguousarray(q[:, :, 0:128].transpose(0, 2, 1), np.float16)
    vo = np.ascontiguousarray(q[:, :, 128:256].transpose(0, 2, 1), np.float16)
    shared = {**mats, **wts}
    in_maps = []
    for c in range(N_CORES):
        m = dict(shared)
        m["xt"] = np.ascontiguousarray(xt[BPC * c:BPC * (c + 1)])
        m["ve"] = np.ascontiguousarray(ve[BPC * c:BPC * (c + 1)])
        m["vo"] = np.ascontiguousarray(vo[BPC * c:BPC * (c + 1)])
        in_maps.append(m)
    return in_maps


def kernel(**inputs):
    nc = build_nc(loop_n=1)
    in_maps = _make_in_maps(inputs)
    res = run_bass_kernel_spmd(nc, in_maps, list(range(N_CORES)))
    out = np.concatenate([r["out"].reshape(BPC, 1, T * HOP)
                          for r in res.results], axis=0)
    return out.astype(np.float32)
